# revision 6
# baseline (speedup 1.0000x reference)
"""GAT message-passing layer (segment softmax + weighted scatter) on 8 trn2 cores.

Strategy: 1D-partition destination nodes across the 8 cores (1250 each); every
edge is routed to the core that owns its destination (the sharding hint's
"partition src_idx/dst_idx/messages" option), so cores run independently with
no collectives.

Host-side prep (index planning + data layout only): destinations are packed
into nw=48 windows of <=27 rows each (degree-balanced LPT), edges are slotted
into T tiles of 128 per window, and the per-edge message rows (source features
pre-scaled elementwise by w_src, bf16) are laid out in slot order so the
device reads them as a single contiguous stream -- no per-edge DMA descriptors
anywhere.  A per-slot one-hot over the window rows is also host-built.

Device-side per macro-chunk of 6 windows (84 tiles):
  - stream the message rows + one-hot,
  - per-edge logit s = row-sum of the pre-scaled row (2 bf16 tree-add levels
    at 2 elem/cyc on DVE + one 1x tensor_reduce),
  - t = s + s_dst (Pool engine, broadcast add), x = exp(leaky_relu(t))
    (Activation engine, full [tile x row] cross),
  - X = onehot * x (DVE, bf16 2x), then per tile a [128edge x 27dst] x
    [128edge x 128feat] PE matmul accumulates features and a second 1-column
    matmul accumulates the softmax denominator, both in PSUM,
  - on close: out = num * recip(den + empty_mask), then * mask/w_src[c]
    (un-scales the pre-scaled features) + h_type on isolated nodes.
"""

import math
import os
import sys

import numpy as np

for _p in ("/opt/trn_rl_repo", "/root/.axon_site/_ro/trn_rl_repo"):
    if os.path.isdir(_p) and _p not in sys.path:
        sys.path.insert(0, _p)

import ml_dtypes  # noqa: E402

import concourse.bacc as bacc  # noqa: E402
import concourse.bass as bass  # noqa: E402
import concourse.mybir as mybir  # noqa: E402
import concourse.tile as tile  # noqa: E402

F32 = mybir.dt.float32
BF16 = mybir.dt.bfloat16
BF = ml_dtypes.bfloat16

N_SENT = 100000
N_TYPE = 10000
D = 128
N_CORES = 8
LEAKY = 0.01

P = 128          # SBUF partitions (edge slots per tile)
W = 27           # destination rows per window (PSUM partition dim)
NW = 48          # windows per core
MC = 6           # windows per macro-chunk
WG = 3           # windows per feature-PSUM tile (3*128 <= 512 f32)


def _plan(src_idx, dst_idx, n_type=N_TYPE, n_cores=N_CORES):
    """Window assignment + edge slotting. Integer index work only."""
    dpc = n_type // n_cores
    deg = np.bincount(dst_idx, minlength=n_type)
    wof = np.empty(n_type, np.int64)
    rof = np.empty(n_type, np.int64)
    loads_all = np.zeros((n_cores, NW), np.int64)
    for c in range(n_cores):
        base = c * dpc
        counts = np.zeros(NW, np.int64)
        loads = np.zeros(NW, np.int64)
        for dl in np.argsort(-deg[base:base + dpc], kind="stable"):
            elig = np.where(counts < W, loads, np.iinfo(np.int64).max)
            w = int(np.argmin(elig))
            wof[base + dl] = w
            rof[base + dl] = counts[w]
            counts[w] += 1
            loads[w] += deg[base + dl]
        loads_all[c] = loads
    T = max(14, int(-(-loads_all.max() // P)))
    spw = T * P                       # slots per window
    nslots = NW * spw                 # per core

    # slot of each edge: edges grouped by (core, window), any order within
    dsti = dst_idx.astype(np.int64)
    core_of = dsti // dpc
    gkey = core_of * NW + wof[dsti]
    order = np.argsort(gkey, kind="stable")
    gcnt = np.bincount(gkey, minlength=n_cores * NW)
    gstart = np.zeros(n_cores * NW + 1, np.int64)
    gstart[1:] = np.cumsum(gcnt)
    slot = np.empty(len(order), np.int64)   # slot within the core, edge-order
    pos_in_g = np.arange(len(order)) - gstart[gkey[order]]
    slot[order] = (gkey[order] % NW) * spw + pos_in_g

    return {"dpc": dpc, "T": T, "deg": deg, "wof": wof, "rof": rof,
            "order": order, "slot": slot, "nslots": nslots}


def _in_maps(plan, h_sent, h_type, attn_w, src_idx, dst_idx):
    dpc, T, nslots = plan["dpc"], plan["T"], plan["nslots"]
    wof, rof, deg = plan["wof"], plan["rof"], plan["deg"]
    ntiles = NW * T
    w1 = attn_w[0, :D].astype(np.float32)
    w2 = attn_w[0, D:].astype(np.float32)
    assert np.abs(w1).min() > 1e-20
    hw16 = (h_sent * w1).astype(BF)            # pre-scaled message rows
    recw1 = (1.0 / w1).astype(np.float32)

    maps = []
    for c in range(N_CORES):
        base = c * dpc
        emask = (src_idx[plan["order"]] >= 0)  # edges of this core, in order
        sel = plan["order"][(dst_idx[plan["order"]] // dpc) == c]
        slots = plan["slot"][sel]
        p_of = slots % P
        t_of = slots // P

        etab = np.zeros((P, ntiles * D), BF)
        etab_v = etab.reshape(P, ntiles, D)
        etab_v[p_of, t_of] = hw16[src_idx[sel]]
        oh = np.zeros((P, ntiles * W), BF)
        oh_v = oh.reshape(P, ntiles, W)
        oh_v[p_of, t_of, rof[dst_idx[sel]]] = 1.0

        # window-layout destination tables [W, NW*D]
        dl = np.arange(base, base + dpc)
        r_l, w_l = rof[dl], wof[dl]
        sdht = np.zeros((W, NW, D), np.float32)
        sdht[r_l, w_l] = h_type[dl]
        mask = np.zeros((W, NW), np.float32)
        mask[r_l, w_l] = (deg[dl] > 0).astype(np.float32)
        imask = np.zeros((W, NW), np.float32)
        imask[r_l, w_l] = (deg[dl] == 0).astype(np.float32)
        imask[mask + imask == 0] = 1.0         # unused (w, r) slots
        htm = (sdht * imask[:, :, None]).astype(BF)
        mwc = (mask[:, :, None] * recw1[None, None, :]).astype(BF)
        w2rep = np.ascontiguousarray(np.broadcast_to(w2.astype(BF), (W, D)))

        maps.append({
            "etab": etab, "oh": oh,
            "sdht": sdht.reshape(W, NW * D).astype(BF),
            "w2rep": w2rep,
            "imask": np.ascontiguousarray(imask),
            "mwc": np.ascontiguousarray(mwc.reshape(W, NW * D)),
            "htm": np.ascontiguousarray(htm.reshape(W, NW * D)),
        })
    return maps


def _build(plan):
    T = plan["T"]
    ntiles = NW * T
    TM = MC * T                     # tiles per macro-chunk
    NMC = NW // MC
    A = mybir.AluOpType

    nc = bacc.Bacc(None, target_bir_lowering=False, debug=False)
    etab_d = nc.dram_tensor("etab", [P, ntiles * D], BF16, kind="ExternalInput")
    oh_d = nc.dram_tensor("oh", [P, ntiles * W], BF16, kind="ExternalInput")
    sdht_d = nc.dram_tensor("sdht", [W, NW * D], BF16, kind="ExternalInput")
    w2_d = nc.dram_tensor("w2rep", [W, D], BF16, kind="ExternalInput")
    imask_d = nc.dram_tensor("imask", [W, NW], F32, kind="ExternalInput")
    mwc_d = nc.dram_tensor("mwc", [W, NW * D], BF16, kind="ExternalInput")
    htm_d = nc.dram_tensor("htm", [W, NW * D], BF16, kind="ExternalInput")
    out_d = nc.dram_tensor("out_local", [NW * W, D], F32, kind="ExternalOutput")
    sd_scr = nc.dram_tensor("sd_scratch", [1, NW * W], F32)

    with tile.TileContext(nc) as tc:
        with (
            tc.tile_pool(name="const", bufs=1) as const,
            tc.tile_pool(name="work", bufs=2) as work,
            tc.tile_pool(name="scratch", bufs=1) as scratch,
            tc.tile_pool(name="psum", bufs=2, space="PSUM") as psum,
        ):
            # ---- consts ----
            sdht = const.tile([W, NW * D], BF16)
            nc.sync.dma_start(out=sdht[:], in_=sdht_d[:, :])
            w2t = const.tile([W, D], BF16)
            nc.sync.dma_start(out=w2t[:], in_=w2_d[:, :])
            imask = const.tile([W, NW], F32)
            nc.sync.dma_start(out=imask[:], in_=imask_d[:, :])
            mwc = const.tile([W, NW * D], BF16)
            nc.sync.dma_start(out=mwc[:], in_=mwc_d[:, :])
            htm = const.tile([W, NW * D], BF16)
            nc.sync.dma_start(out=htm[:], in_=htm_d[:, :])
            ones1 = const.tile([P, 1], BF16)
            nc.vector.memset(ones1[:], 1.0)

            # ---- s_dst per (row, window), replicated to 128 partitions ----
            sd3 = sdht[:].rearrange("p (w f) -> p w f", f=D)
            sdtmp = scratch.tile([W, NW * D], BF16)
            sdtmp3 = sdtmp[:].rearrange("p (w f) -> p w f", f=D)
            w2b = w2t[:].rearrange("p (a f) -> p a f", a=1).to_broadcast([W, NW, D])
            nc.vector.tensor_tensor(out=sdtmp3, in0=sd3, in1=w2b, op=A.mult)
            sd = scratch.tile([W, NW], F32)
            nc.vector.tensor_reduce(out=sd[:], in_=sdtmp3, axis=mybir.AxisListType.X,
                                    op=A.add)
            nc.sync.dma_start(
                out=sd_scr[0, 0:NW * W].rearrange("(w r) -> r w", r=W), in_=sd[:, :])
            sdrow = scratch.tile([1, NW * W], F32)
            nc.sync.dma_start(out=sdrow[:], in_=sd_scr[0:1, :])
            onesf = scratch.tile([1, P], F32)
            nc.vector.memset(onesf[:], 1.0)
            sdrep = const.tile([P, NW * W], F32)
            for i in range(math.ceil(NW * W / 512)):
                n = min(512, NW * W - i * 512)
                pt = psum.tile([P, 512], F32, tag="rep")
                nc.tensor.matmul(out=pt[:, 0:n], lhsT=onesf[:],
                                 rhs=sdrow[:, i * 512:i * 512 + n],
                                 start=True, stop=True)
                nc.vector.tensor_copy(out=sdrep[:, i * 512:i * 512 + n], in_=pt[:, 0:n])

            numbuf = const.tile([W, NW * D], F32)

            # ---- main loop over macro-chunks ----
            for mc in range(NMC):
                t0 = mc * TM
                hbuf = work.tile([P, TM * D], BF16, tag="hbuf")
                qs = 4
                for q in range(qs):
                    sl = TM * D // qs
                    nc.sync.dma_start(
                        out=hbuf[:, q * sl:(q + 1) * sl],
                        in_=etab_d[:, t0 * D + q * sl: t0 * D + (q + 1) * sl])
                ohb = work.tile([P, TM * W], BF16, tag="ohb")
                for q in range(2):
                    sl = TM * W // 2
                    nc.sync.dma_start(
                        out=ohb[:, q * sl:(q + 1) * sl],
                        in_=oh_d[:, t0 * W + q * sl: t0 * W + (q + 1) * sl])

                # s = row-sum of pre-scaled rows (2 tree levels + reduce)
                hb4 = hbuf[:].rearrange("p (t u f) -> p t u f", u=2, f=64)
                sL1 = work.tile([P, TM * 64], BF16, tag="sL1")
                sL13 = sL1[:].rearrange("p (t f) -> p t f", f=64)
                nc.vector.tensor_tensor(out=sL13, in0=hb4[:, :, 0, :],
                                        in1=hb4[:, :, 1, :], op=A.add)
                sL14 = sL1[:].rearrange("p (t u f) -> p t u f", u=2, f=32)
                sL2 = work.tile([P, TM * 32], BF16, tag="sL2")
                sL23 = sL2[:].rearrange("p (t f) -> p t f", f=32)
                nc.vector.tensor_tensor(out=sL23, in0=sL14[:, :, 0, :],
                                        in1=sL14[:, :, 1, :], op=A.add)
                scol = work.tile([P, TM], F32, tag="scol")
                nc.vector.tensor_reduce(out=scol[:], in_=sL23,
                                        axis=mybir.AxisListType.X, op=A.add)

                # t = s + s_dst  (Pool), x = exp(leaky(t)) (Activation)
                tfull = work.tile([P, TM * W], F32, tag="tfull")
                tfull4 = tfull[:].rearrange("p (w t r) -> p w t r", w=MC, r=W)
                scol4 = (scol[:].rearrange("p (w t) -> p w t", w=MC)
                         .rearrange("p w (t a) -> p w t a", a=1)
                         .to_broadcast([P, MC, T, W]))
                sdrep4 = (sdrep[:, mc * MC * W:(mc + 1) * MC * W]
                          .rearrange("p (w r) -> p w r", r=W)
                          .rearrange("p w (a r) -> p w a r", a=1)
                          .to_broadcast([P, MC, T, W]))
                nc.gpsimd.tensor_tensor(out=tfull4, in0=scol4, in1=sdrep4, op=A.add)
                # exp(leaky_relu(t)) == max(exp(t), exp(0.01*t)) exactly
                xfull = work.tile([P, TM * W], BF16, tag="xfull")
                nc.scalar.activation(out=xfull[:], in_=tfull[:],
                                     func=mybir.ActivationFunctionType.Exp)
                x01 = work.tile([P, TM * W], BF16, tag="x01")
                nc.scalar.activation(out=x01[:], in_=tfull[:],
                                     func=mybir.ActivationFunctionType.Exp,
                                     scale=LEAKY)
                nc.vector.tensor_tensor(out=xfull[:], in0=xfull[:], in1=x01[:],
                                        op=A.max)

                # X = onehot * x (in place over the one-hot)
                nc.vector.tensor_tensor(out=ohb[:], in0=ohb[:], in1=xfull[:],
                                        op=A.mult)
                X3 = ohb[:].rearrange("p (t r) -> p t r", r=W)
                hb3 = hbuf[:].rearrange("p (t f) -> p t f", f=D)

                # PE scatter: features + denominator
                fps = []
                for k in range(MC // WG):
                    fpt = psum.tile([W, WG * D], F32, tag=f"fp{k}", name=f"fp{k}")
                    fps.append(fpt)
                dps = psum.tile([W, MC], F32, tag="dp")
                for wl in range(MC):
                    fp = fps[wl // WG]
                    c0 = (wl % WG) * D
                    for j in range(T):
                        t = wl * T + j
                        nc.tensor.matmul(out=fp[:, c0:c0 + D], lhsT=X3[:, t, :],
                                         rhs=hb3[:, t, :], start=(j == 0),
                                         stop=(j == T - 1))
                        nc.tensor.matmul(out=dps[:, wl:wl + 1], lhsT=X3[:, t, :],
                                         rhs=ones1[:], start=(j == 0),
                                         stop=(j == T - 1))

                # close: num/den (+ guard for empty rows)
                w0 = mc * MC
                dadj = work.tile([W, MC], F32, tag="dadj")
                nc.vector.tensor_tensor(out=dadj[:], in0=dps[:],
                                        in1=imask[:, w0:w0 + MC], op=A.add)
                rec = work.tile([W, MC], F32, tag="rec")
                nc.vector.reciprocal(out=rec[:], in_=dadj[:])
                for k in range(MC // WG):
                    nb = (numbuf[:, (w0 + k * WG) * D:(w0 + (k + 1) * WG) * D]
                          .rearrange("p (w f) -> p w f", f=D))
                    rb = (rec[:, k * WG:(k + 1) * WG]
                          .rearrange("p (w a) -> p w a", a=1)
                          .to_broadcast([W, WG, D]))
                    fp3 = fps[k][:].rearrange("p (w f) -> p w f", f=D)
                    nc.vector.tensor_tensor(out=nb, in0=fp3, in1=rb, op=A.mult)

            # ---- final blend + un-scale ----
            nc.gpsimd.tensor_tensor(out=numbuf[:], in0=numbuf[:], in1=mwc[:],
                                    op=A.mult)
            nc.vector.tensor_tensor(out=numbuf[:], in0=numbuf[:], in1=htm[:],
                                    op=A.add)
            nc.sync.dma_start(
                out=out_d[:, :].rearrange("(w r) f -> r w f", r=W),
                in_=numbuf[:].rearrange("p (w f) -> p w f", f=D))

    nc.finalize()
    return nc


def prepare(h_sent, h_type, attn_w, src_idx, dst_idx):
    plan = _plan(np.asarray(src_idx), np.asarray(dst_idx))
    nc = _build(plan)
    maps = _in_maps(plan, np.asarray(h_sent, dtype=np.float32),
                    np.asarray(h_type, dtype=np.float32),
                    np.asarray(attn_w, dtype=np.float32),
                    np.asarray(src_idx), np.asarray(dst_idx))
    return plan, nc, maps


def unpermute(plan, results):
    dpc = plan["dpc"]
    out = np.empty((N_CORES * dpc, D), np.float32)
    for c in range(N_CORES):
        rows = results[c]["out_local"]
        base = c * dpc
        dl = np.arange(base, base + dpc)
        rowpos = plan["wof"][dl] * W + plan["rof"][dl]
        out[base:base + dpc] = rows[rowpos]
    return out


def kernel(h_sent, h_type, attn_w, src_idx, dst_idx):
    from concourse.bass_utils import run_bass_kernel_spmd

    plan, nc, maps = prepare(h_sent, h_type, attn_w, src_idx, dst_idx)
    res = run_bass_kernel_spmd(nc, maps, list(range(N_CORES)))
    return unpermute(plan, res.results)
